# revision 11
# baseline (speedup 1.0000x reference)
"""MoE layer (E=8 experts, top-2, D=1024, H=4096, N=4096 tokens) on 8 TRN2
NeuronCores.

Strategy: expert-parallel. The router (gate matmul + softmax + top-2) is a
negligible ~0.1% of the FLOPs, so it runs on host in float64 (verified to
reproduce the reference's f32 top-2 selection exactly). The host gathers
each expert's tokens, pads them to a common capacity C (max expert load
rounded up to the chunk size), and ships one expert per core. Each core
runs a dense single-expert FFN over its C tokens:

    outT = w2.T @ gelu(w1.T @ xT + b1)

i.e. 2*C*D*H MACs/core instead of the 8*2*T*D*H of the dense data-parallel
formulation — a ~3.6x FLOP cut (only top-2 experts per token are computed).
The host applies the combine weights and the (comb-weighted) b2 bias during
the scatter-add back to token order.

Activations keep the feature dim on partitions (tokens on the free axis) so
both matmuls consume pre-tiled weights with no on-device transposes:
    hT[h, t]   = sum_d w1[d, h] * xT[d, t]     (lhsT = w1 tile, rhs = xT)
    outT[d, t] = sum_h w2[h, d] * geluT[h, t]  (lhsT = w2 tile, rhs = geluT)

Matmuls run in bf16 (1 PE row/cycle, FWL-enabled weight loads) with fp32
PSUM accumulation; measured end-to-end error vs the f32 reference is
~3.4e-3 absmax-relative, well under the 2e-2 gate. Expected PE-bound time:
2 * 8 * 32 * C cycles @ 2.4 GHz ~= 246 us for C=1152.
"""

import numpy as np
import ml_dtypes

import concourse.bass as bass  # noqa: F401  (bass types used via tile/bacc)
import concourse.mybir as mybir
import concourse.tile as tile
from concourse import bacc, bass_utils

F32 = mybir.dt.float32
BF16 = mybir.dt.bfloat16
AFT = mybir.ActivationFunctionType
NPBF16 = np.dtype(ml_dtypes.bfloat16)

E = 8          # experts (== cores; expert e runs on core e)
D = 1024       # model dim
H = 4096       # expert hidden dim
P = 128        # partitions
NCORES = 8
NTOK = 4096    # total tokens (B*T = 2*2048)
KD = D // P    # 8 contraction chunks of D
NH = H // P    # 32 h tiles
ND = D // P    # 8 d tiles
NCH = 3        # token chunks per core (chunk = psum free dim, <= 512 f32/bank)
NWARM = 36     # dummy matmuls at t=0: warm the PE clock (HAM) during DMA fill

_NC = {}       # compiled modules keyed by (nch, ch)


def _build(nch, ch):
    c = nch * ch   # per-core token capacity
    nc = bacc.Bacc("TRN2", target_bir_lowering=False, debug=False,
                   num_devices=NCORES)
    xT = nc.dram_tensor("xT", [P, KD, c], BF16, kind="ExternalInput").ap()
    w1t = nc.dram_tensor("w1t", [NH, P, KD, P], BF16,
                         kind="ExternalInput").ap()
    b1t = nc.dram_tensor("b1t", [P, NH], F32, kind="ExternalInput").ap()
    w2t = nc.dram_tensor("w2t", [ND, P, NH, P], BF16,
                         kind="ExternalInput").ap()
    outT = nc.dram_tensor("outT", [P, ND, c], F32, kind="ExternalOutput").ap()

    with tile.TileContext(nc) as tc:
        with (
            tc.tile_pool(name="const", bufs=1) as cpool,
            tc.tile_pool(name="w1p", bufs=4) as w1p,
            tc.tile_pool(name="w2p", bufs=2) as w2p,
            tc.tile_pool(name="otp", bufs=4) as otp,
            tc.tile_pool(name="ps", bufs=7, space="PSUM") as ps,
            tc.tile_pool(name="pw", bufs=1, space="PSUM") as pw,
        ):
            # ---- PE warmup: dummy matmuls with no DMA dependency keep the
            # HAM activity window busy during the input fill, so real
            # matmuls start at 2.4 GHz instead of ramping from 1.2.
            wsrc = cpool.tile([P, 640], BF16)
            nc.gpsimd.memset(wsrc[:], 0.0)
            pwt = pw.tile([P, 512], F32)
            for _ in range(NWARM):
                nc.tensor.matmul(pwt[:], wsrc[:, 0:P], wsrc[:, P:640],
                                 start=True, stop=True)

            # ---- persistent SBUF; first mm needs w1[0] (sync queue) + the
            # chunk-0 columns of xT (scalar queue, issues in parallel)
            w1_first = w1p.tile([P, KD, P], BF16)
            nc.sync.dma_start(w1_first[:], w1t[0])
            b1_s = cpool.tile([P, NH], F32)
            nc.sync.dma_start(b1_s[:], b1t[:])
            xTt = cpool.tile([P, KD, c], BF16)
            for t in range(nch):
                ts = slice(t * ch, (t + 1) * ch)
                for kd in range(KD):
                    nc.scalar.dma_start(xTt[:, kd, ts], xT[:, kd, ts])
            geluT = cpool.tile([P, NH, c], BF16)

            # ---- mm1 + gelu: geluT[h, t] = gelu(sum_d w1[d,h] x[d,t] + b1)
            for h in range(NH):
                if h == 0:
                    w1_s = w1_first
                else:
                    w1_s = w1p.tile([P, KD, P], BF16)
                    nc.sync.dma_start(w1_s[:], w1t[h])
                for t in range(nch):
                    ts = slice(t * ch, (t + 1) * ch)
                    ph = ps.tile([P, ch], F32, tag="ps")
                    for kd in range(KD):
                        nc.tensor.matmul(ph[:], w1_s[:, kd, :],
                                         xTt[:, kd, ts],
                                         start=(kd == 0), stop=(kd == KD - 1))
                    nc.scalar.activation(geluT[:, h, ts], ph[:], AFT.Gelu,
                                         bias=b1_s[:, h:h + 1])

            # ---- mm2: outT[d, t] = sum_h w2[h,d] geluT[h,t]
            for d in range(ND):
                w2_s = w2p.tile([P, NH, P], BF16)
                for q in range(2):
                    nc.sync.dma_start(
                        w2_s[:, q * NH // 2:(q + 1) * NH // 2, :],
                        w2t[d, :, q * NH // 2:(q + 1) * NH // 2, :])
                for t in range(nch):
                    ts = slice(t * ch, (t + 1) * ch)
                    po = ps.tile([P, ch], F32, tag="ps")
                    for hh in range(NH):
                        nc.tensor.matmul(po[:], w2_s[:, hh, :],
                                         geluT[:, hh, ts],
                                         start=(hh == 0), stop=(hh == NH - 1))
                    ot = otp.tile([P, ch], F32)
                    nc.vector.tensor_copy(ot[:], po[:])
                    nc.sync.dma_start(outT[:, d, ts], ot[:])

    nc.compile()
    return nc


def _get_nc(nch, ch):
    if (nch, ch) not in _NC:
        _NC[(nch, ch)] = _build(nch, ch)
    return _NC[(nch, ch)]


def _route(xf, gate_w, gate_b):
    """Top-2 routing in float64 (reproduces the reference's f32 decisions)."""
    lg = xf.astype(np.float64) @ gate_w.astype(np.float64) \
        + gate_b.astype(np.float64)
    lg -= lg.max(-1, keepdims=True)
    g = np.exp(lg)
    g /= g.sum(-1, keepdims=True)
    ti = np.argsort(-g, axis=-1, kind="stable")[:, :2]     # [N, 2] desc
    tg = np.take_along_axis(g, ti, axis=1)
    tg = tg / (tg.sum(-1, keepdims=True) + 1e-9)           # combine weights
    return ti, tg


def _prep(x, gate_w, gate_b, w1, b1, w2, b2):
    f = np.float32
    xf = np.asarray(x, f).reshape(NTOK, D)
    gate_w = np.asarray(gate_w, f)
    gate_b = np.asarray(gate_b, f)
    w1 = np.asarray(w1, f)
    b1 = np.asarray(b1, f)
    w2 = np.asarray(w2, f)
    b2 = np.asarray(b2, f)

    ti, tg = _route(xf, gate_w, gate_b)

    sels, wts = [], []
    for e in range(E):
        m = (ti == e)
        sel = np.nonzero(m.any(1))[0]                       # token ids, asc
        wt = tg[sel, m[sel].argmax(1)].astype(f)            # combine weight
        sels.append(sel)
        wts.append(wt)
    maxc = max(len(s) for s in sels)
    ch = -(-maxc // (NCH * 4)) * 4       # chunk size, 4-aligned
    nch = NCH
    if ch > 512:                          # capacity > 1536 tokens: more chunks
        ch = 512
        nch = -(-maxc // ch)
    c = nch * ch

    in_maps = []
    for e in range(E):
        sel = sels[e]
        xe = np.zeros((c, D), f)
        xe[:len(sel)] = xf[sel]
        xTe = np.ascontiguousarray(
            xe.T.reshape(KD, P, c).transpose(1, 0, 2)).astype(NPBF16)
        w1te = np.ascontiguousarray(
            w1[e].reshape(KD, P, NH, P).transpose(2, 1, 0, 3)).astype(NPBF16)
        b1te = np.ascontiguousarray(b1[e].reshape(NH, P).T)
        w2te = np.ascontiguousarray(
            w2[e].reshape(NH, P, ND, P).transpose(2, 1, 0, 3)).astype(NPBF16)
        in_maps.append({"xT": xTe, "w1t": w1te, "b1t": b1te, "w2t": w2te})
    return in_maps, sels, wts, b2, nch, ch


def _assemble(results, sels, wts, b2):
    out = np.zeros((NTOK, D), np.float32)
    for e in range(E):
        sel = sels[e]
        yT = np.asarray(results[e]["outT"])                 # [P, ND, c] f32
        y = yT.transpose(1, 0, 2).reshape(D, -1).T          # [c, D]
        out[sel] += wts[e][:, None] * (y[:len(sel)] + b2[e])
    return out.reshape(2, NTOK // 2, D)


def run(inputs, trace=False):
    """Run the kernel; returns (output, exec_time_ns or None)."""
    in_maps, sels, wts, b2, nch, ch = _prep(**inputs)
    nc = _get_nc(nch, ch)
    res = bass_utils.run_bass_kernel_spmd(
        nc, in_maps, core_ids=list(range(NCORES)), trace=trace)
    return _assemble(res.results, sels, wts, b2), res.exec_time_ns


def kernel(**inputs):
    out, _ = run(inputs, trace=False)
    return out


# revision 14
# speedup vs baseline: 1.1860x; 1.1860x over previous
"""MoE layer (E=8 experts, top-2, D=1024, H=4096, N=4096 tokens) on 8 TRN2
NeuronCores.

Strategy: expert-parallel. The router (gate matmul + softmax + top-2) is a
negligible ~0.1% of the FLOPs, so it runs on host in float64 (verified to
reproduce the reference's f32 top-2 selection exactly). The host gathers
each expert's tokens, pads them to a common capacity C (max expert load
rounded up to the chunk size), and ships one expert per core. Each core
runs a dense single-expert FFN over its C tokens:

    outT = w2.T @ gelu(w1.T @ xT + b1)

i.e. 2*C*D*H MACs/core instead of the 8*2*T*D*H of the dense data-parallel
formulation — a ~3.6x FLOP cut (only top-2 experts per token are computed).
The host applies the combine weights and the (comb-weighted) b2 bias during
the scatter-add back to token order.

Activations keep the feature dim on partitions (tokens on the free axis) so
both matmuls consume pre-tiled weights with no on-device transposes:
    hT[h, t]   = sum_d w1[d, h] * xT[d, t]     (lhsT = w1 tile, rhs = xT)
    outT[d, t] = sum_h w2[h, d] * geluT[h, t]  (lhsT = w2 tile, rhs = geluT)

Matmuls run in bf16 (1 PE row/cycle, FWL-enabled weight loads) with fp32
PSUM accumulation; measured end-to-end error vs the f32 reference is
~3.4e-3 absmax-relative, well under the 2e-2 gate. Expected PE-bound time:
2 * 8 * 32 * C cycles @ 2.4 GHz ~= 246 us for C=1152.
"""

import numpy as np
import ml_dtypes

import concourse.bass as bass  # noqa: F401  (bass types used via tile/bacc)
import concourse.mybir as mybir
import concourse.tile as tile
from concourse import bacc, bass_utils

F32 = mybir.dt.float32
BF16 = mybir.dt.bfloat16
AFT = mybir.ActivationFunctionType
NPBF16 = np.dtype(ml_dtypes.bfloat16)

E = 8          # experts (== cores; expert e runs on core e)
D = 1024       # model dim
H = 4096       # expert hidden dim
P = 128        # partitions
NCORES = 8
NTOK = 4096    # total tokens (B*T = 2*2048)
KD = D // P    # 8 contraction chunks of D
NH = H // P    # 32 h tiles
ND = D // P    # 8 d tiles
NCH = 3        # token chunks per core (chunk = psum free dim, <= 512 f32/bank)

_NC = {}       # compiled modules keyed by (nch, ch)


def _build(nch, ch):
    c = nch * ch   # per-core token capacity
    nc = bacc.Bacc("TRN2", target_bir_lowering=False, debug=False,
                   num_devices=NCORES)
    xT = nc.dram_tensor("xT", [P, KD, c], BF16, kind="ExternalInput").ap()
    w1t = nc.dram_tensor("w1t", [NH, P, KD, P], BF16,
                         kind="ExternalInput").ap()
    b1t = nc.dram_tensor("b1t", [P, NH], F32, kind="ExternalInput").ap()
    w2t = nc.dram_tensor("w2t", [ND, P, NH, P], BF16,
                         kind="ExternalInput").ap()
    outT = nc.dram_tensor("outT", [P, ND, c], F32, kind="ExternalOutput").ap()

    with tile.TileContext(nc) as tc:
        with (
            tc.tile_pool(name="const", bufs=1) as cpool,
            tc.tile_pool(name="w1p", bufs=4) as w1p,
            tc.tile_pool(name="w2p", bufs=2) as w2p,
            tc.tile_pool(name="otp", bufs=4) as otp,
            tc.tile_pool(name="ps", bufs=8, space="PSUM") as ps,
        ):
            # ---- persistent SBUF. DMA *instructions* issue serially at
            # ~0.65us each per issuing queue, so the emission order and the
            # sync/scalar queue split below are what set the time-to-first-
            # matmul: sync issues w1[0..1] while scalar issues the chunk-0
            # columns of xT.
            w1_pre = []
            for h in range(2):
                w1_s = w1p.tile([P, KD, P], BF16)
                nc.sync.dma_start(w1_s[:], w1t[h])
                w1_pre.append(w1_s)
            xTt = cpool.tile([P, KD, c], BF16)
            for t in range(nch):
                ts = slice(t * ch, (t + 1) * ch)
                for kd in range(KD):
                    nc.scalar.dma_start(xTt[:, kd, ts], xT[:, kd, ts])
            b1_s = cpool.tile([P, NH], F32)
            nc.sync.dma_start(b1_s[:], b1t[:])
            geluT = cpool.tile([P, NH, c], BF16)

            # ---- mm1 + gelu: geluT[h, t] = gelu(sum_d w1[d,h] x[d,t] + b1)
            for h in range(NH):
                if h < 2:
                    w1_s = w1_pre[h]
                else:
                    w1_s = w1p.tile([P, KD, P], BF16)
                    nc.sync.dma_start(w1_s[:], w1t[h])
                for t in range(nch):
                    ts = slice(t * ch, (t + 1) * ch)
                    ph = ps.tile([P, ch], F32, tag="ps")
                    for kd in range(KD):
                        nc.tensor.matmul(ph[:], w1_s[:, kd, :],
                                         xTt[:, kd, ts],
                                         start=(kd == 0), stop=(kd == KD - 1))
                    nc.scalar.activation(geluT[:, h, ts], ph[:], AFT.Gelu,
                                         bias=b1_s[:, h:h + 1])

            # ---- mm2: outT[d, t] = sum_h w2[h,d] geluT[h,t]
            for d in range(ND):
                w2_s = w2p.tile([P, NH, P], BF16)
                for q in range(2):
                    nc.sync.dma_start(
                        w2_s[:, q * NH // 2:(q + 1) * NH // 2, :],
                        w2t[d, :, q * NH // 2:(q + 1) * NH // 2, :])
                for t in range(nch):
                    ts = slice(t * ch, (t + 1) * ch)
                    po = ps.tile([P, ch], F32, tag="ps")
                    for hh in range(NH):
                        nc.tensor.matmul(po[:], w2_s[:, hh, :],
                                         geluT[:, hh, ts],
                                         start=(hh == 0), stop=(hh == NH - 1))
                    ot = otp.tile([P, ch], F32)
                    nc.vector.tensor_copy(ot[:], po[:])
                    nc.sync.dma_start(outT[:, d, ts], ot[:])

    nc.compile()
    return nc


def _get_nc(nch, ch):
    if (nch, ch) not in _NC:
        _NC[(nch, ch)] = _build(nch, ch)
    return _NC[(nch, ch)]


def _route(xf, gate_w, gate_b):
    """Top-2 routing in float64 (reproduces the reference's f32 decisions)."""
    lg = xf.astype(np.float64) @ gate_w.astype(np.float64) \
        + gate_b.astype(np.float64)
    lg -= lg.max(-1, keepdims=True)
    g = np.exp(lg)
    g /= g.sum(-1, keepdims=True)
    ti = np.argsort(-g, axis=-1, kind="stable")[:, :2]     # [N, 2] desc
    tg = np.take_along_axis(g, ti, axis=1)
    tg = tg / (tg.sum(-1, keepdims=True) + 1e-9)           # combine weights
    return ti, tg


def _prep(x, gate_w, gate_b, w1, b1, w2, b2):
    f = np.float32
    xf = np.asarray(x, f).reshape(NTOK, D)
    gate_w = np.asarray(gate_w, f)
    gate_b = np.asarray(gate_b, f)
    w1 = np.asarray(w1, f)
    b1 = np.asarray(b1, f)
    w2 = np.asarray(w2, f)
    b2 = np.asarray(b2, f)

    ti, tg = _route(xf, gate_w, gate_b)

    sels, wts = [], []
    for e in range(E):
        m = (ti == e)
        sel = np.nonzero(m.any(1))[0]                       # token ids, asc
        wt = tg[sel, m[sel].argmax(1)].astype(f)            # combine weight
        sels.append(sel)
        wts.append(wt)
    maxc = max(len(s) for s in sels)
    ch = -(-maxc // (NCH * 8)) * 8       # chunk size, 8-aligned (16B in bf16)
    nch = NCH
    if ch > 512:                          # capacity > 1536 tokens: more chunks
        ch = 512
        nch = -(-maxc // ch)
    c = nch * ch

    in_maps = []
    for e in range(E):
        sel = sels[e]
        xe = np.zeros((c, D), f)
        xe[:len(sel)] = xf[sel]
        xTe = np.ascontiguousarray(
            xe.T.reshape(KD, P, c).transpose(1, 0, 2)).astype(NPBF16)
        w1te = np.ascontiguousarray(
            w1[e].reshape(KD, P, NH, P).transpose(2, 1, 0, 3)).astype(NPBF16)
        b1te = np.ascontiguousarray(b1[e].reshape(NH, P).T)
        w2te = np.ascontiguousarray(
            w2[e].reshape(NH, P, ND, P).transpose(2, 1, 0, 3)).astype(NPBF16)
        in_maps.append({"xT": xTe, "w1t": w1te, "b1t": b1te, "w2t": w2te})
    return in_maps, sels, wts, b2, nch, ch


def _assemble(results, sels, wts, b2):
    out = np.zeros((NTOK, D), np.float32)
    for e in range(E):
        sel = sels[e]
        yT = np.asarray(results[e]["outT"])                 # [P, ND, c] f32
        y = yT.transpose(1, 0, 2).reshape(D, -1).T          # [c, D]
        out[sel] += wts[e][:, None] * (y[:len(sel)] + b2[e])
    return out.reshape(2, NTOK // 2, D)


def run(inputs, trace=False):
    """Run the kernel; returns (output, exec_time_ns or None)."""
    in_maps, sels, wts, b2, nch, ch = _prep(**inputs)
    nc = _get_nc(nch, ch)
    res = bass_utils.run_bass_kernel_spmd(
        nc, in_maps, core_ids=list(range(NCORES)), trace=trace)
    return _assemble(res.results, sels, wts, b2), res.exec_time_ns


def kernel(**inputs):
    out, _ = run(inputs, trace=False)
    return out


# revision 15
# speedup vs baseline: 1.2046x; 1.0157x over previous
"""MoE layer (E=8 experts, top-2, D=1024, H=4096, N=4096 tokens) on 8 TRN2
NeuronCores.

Strategy: expert-parallel. The router (gate matmul + softmax + top-2) is a
negligible ~0.1% of the FLOPs, so it runs on host in float64 (verified to
reproduce the reference's f32 top-2 selection exactly). The host gathers
each expert's tokens, pads them to a common capacity C (max expert load
rounded up to the chunk size), and ships one expert per core. Each core
runs a dense single-expert FFN over its C tokens:

    outT = w2.T @ gelu(w1.T @ xT + b1)

i.e. 2*C*D*H MACs/core instead of the 8*2*T*D*H of the dense data-parallel
formulation — a ~3.6x FLOP cut (only top-2 experts per token are computed).
The host applies the combine weights and the (comb-weighted) b2 bias during
the scatter-add back to token order.

Activations keep the feature dim on partitions (tokens on the free axis) so
both matmuls consume pre-tiled weights with no on-device transposes:
    hT[h, t]   = sum_d w1[d, h] * xT[d, t]     (lhsT = w1 tile, rhs = xT)
    outT[d, t] = sum_h w2[h, d] * geluT[h, t]  (lhsT = w2 tile, rhs = geluT)

Matmuls run in bf16 (1 PE row/cycle, FWL-enabled weight loads) with fp32
PSUM accumulation; measured end-to-end error vs the f32 reference is
~3.4e-3 absmax-relative, well under the 2e-2 gate. Expected PE-bound time:
2 * 8 * 32 * C cycles @ 2.4 GHz ~= 246 us for C=1152.
"""

import numpy as np
import ml_dtypes

import concourse.bass as bass  # noqa: F401  (bass types used via tile/bacc)
import concourse.mybir as mybir
import concourse.tile as tile
from concourse import bacc, bass_utils

F32 = mybir.dt.float32
BF16 = mybir.dt.bfloat16
AFT = mybir.ActivationFunctionType
NPBF16 = np.dtype(ml_dtypes.bfloat16)

E = 8          # experts (== cores; expert e runs on core e)
D = 1024       # model dim
H = 4096       # expert hidden dim
P = 128        # partitions
NCORES = 8
NTOK = 4096    # total tokens (B*T = 2*2048)
KD = D // P    # 8 contraction chunks of D
NH = H // P    # 32 h tiles
ND = D // P    # 8 d tiles
NCH = 3        # token chunks per core (chunk = psum free dim, <= 512 f32/bank)

_NC = {}       # compiled modules keyed by (nch, ch)


def _build(nch, ch):
    c = nch * ch   # per-core token capacity
    nc = bacc.Bacc("TRN2", target_bir_lowering=False, debug=False,
                   num_devices=NCORES)
    xT = nc.dram_tensor("xT", [P, KD, c], BF16, kind="ExternalInput").ap()
    w1t = nc.dram_tensor("w1t", [NH, P, KD, P], BF16,
                         kind="ExternalInput").ap()
    b1t = nc.dram_tensor("b1t", [P, NH], F32, kind="ExternalInput").ap()
    w2t = nc.dram_tensor("w2t", [ND, P, NH, P], BF16,
                         kind="ExternalInput").ap()
    outT = nc.dram_tensor("outT", [P, ND, c], F32, kind="ExternalOutput").ap()

    with tile.TileContext(nc) as tc:
        with (
            tc.tile_pool(name="const", bufs=1) as cpool,
            tc.tile_pool(name="w1p", bufs=NH) as w1p,
            tc.tile_pool(name="w2p", bufs=2) as w2p,
            tc.tile_pool(name="otp", bufs=4) as otp,
            tc.tile_pool(name="ps", bufs=7, space="PSUM") as ps,
            tc.tile_pool(name="pw", bufs=1, space="PSUM") as pw,
        ):
            # ---- PE warmup: a short train of dependency-free matmuls keeps
            # the HAM activity window busy from ~0.5us so the real stream
            # starts at 2.4 GHz instead of ramping from 1.2 GHz. The train is
            # sized to end just before the first real matmul's data lands.
            wsrc = cpool.tile([P, 256], BF16)
            nc.vector.memset(wsrc[:], 0.0)
            pwt = pw.tile([P, P], F32)
            for _ in range(34):
                nc.tensor.matmul(pwt[:], wsrc[:, 0:P], wsrc[:, P:256],
                                 start=True, stop=True)

            # ---- persistent SBUF. DMA *instructions* issue serially at
            # ~0.65us each per issuing queue, so emission order and the
            # sync/scalar queue split set the time to first matmul: sync
            # issues b1+w1 tiles while scalar issues xT chunk columns. All
            # NH w1 tiles stay resident (64KB/partition) so mm1 can run
            # chunk-major: chunk 1/2 input data isn't touched until ~40us
            # in, by which time the xT fill has long completed.
            b1_s = cpool.tile([P, NH], F32)
            nc.sync.dma_start(b1_s[:], b1t[:])
            w1_all = []
            for h in range(NH):
                w1_s = w1p.tile([P, KD, P], BF16)
                nc.sync.dma_start(w1_s[:], w1t[h])
                w1_all.append(w1_s)
            xTt = cpool.tile([P, KD, c], BF16)
            for t in range(nch):
                ts = slice(t * ch, (t + 1) * ch)
                for kd in range(KD):
                    nc.scalar.dma_start(xTt[:, kd, ts], xT[:, kd, ts])
            geluT = cpool.tile([P, NH, c], BF16)

            # ---- mm1 + gelu: geluT[h, t] = gelu(sum_d w1[d,h] x[d,t] + b1)
            for t in range(nch):
                ts = slice(t * ch, (t + 1) * ch)
                for h in range(NH):
                    ph = ps.tile([P, ch], F32, tag="ps")
                    for kd in range(KD):
                        nc.tensor.matmul(ph[:], w1_all[h][:, kd, :],
                                         xTt[:, kd, ts],
                                         start=(kd == 0), stop=(kd == KD - 1))
                    nc.scalar.activation(geluT[:, h, ts], ph[:], AFT.Gelu,
                                         bias=b1_s[:, h:h + 1])

            # ---- mm2: outT[d, t] = sum_h w2[h,d] geluT[h,t]
            for d in range(ND):
                w2_s = w2p.tile([P, NH, P], BF16)
                for q in range(2):
                    nc.sync.dma_start(
                        w2_s[:, q * NH // 2:(q + 1) * NH // 2, :],
                        w2t[d, :, q * NH // 2:(q + 1) * NH // 2, :])
                for t in range(nch):
                    ts = slice(t * ch, (t + 1) * ch)
                    po = ps.tile([P, ch], F32, tag="ps")
                    for hh in range(NH):
                        nc.tensor.matmul(po[:], w2_s[:, hh, :],
                                         geluT[:, hh, ts],
                                         start=(hh == 0), stop=(hh == NH - 1))
                    ot = otp.tile([P, ch], F32)
                    nc.vector.tensor_copy(ot[:], po[:])
                    nc.sync.dma_start(outT[:, d, ts], ot[:])

    nc.compile()
    return nc


def _get_nc(nch, ch):
    if (nch, ch) not in _NC:
        _NC[(nch, ch)] = _build(nch, ch)
    return _NC[(nch, ch)]


def _route(xf, gate_w, gate_b):
    """Top-2 routing in float64 (reproduces the reference's f32 decisions)."""
    lg = xf.astype(np.float64) @ gate_w.astype(np.float64) \
        + gate_b.astype(np.float64)
    lg -= lg.max(-1, keepdims=True)
    g = np.exp(lg)
    g /= g.sum(-1, keepdims=True)
    ti = np.argsort(-g, axis=-1, kind="stable")[:, :2]     # [N, 2] desc
    tg = np.take_along_axis(g, ti, axis=1)
    tg = tg / (tg.sum(-1, keepdims=True) + 1e-9)           # combine weights
    return ti, tg


def _prep(x, gate_w, gate_b, w1, b1, w2, b2):
    f = np.float32
    xf = np.asarray(x, f).reshape(NTOK, D)
    gate_w = np.asarray(gate_w, f)
    gate_b = np.asarray(gate_b, f)
    w1 = np.asarray(w1, f)
    b1 = np.asarray(b1, f)
    w2 = np.asarray(w2, f)
    b2 = np.asarray(b2, f)

    ti, tg = _route(xf, gate_w, gate_b)

    sels, wts = [], []
    for e in range(E):
        m = (ti == e)
        sel = np.nonzero(m.any(1))[0]                       # token ids, asc
        wt = tg[sel, m[sel].argmax(1)].astype(f)            # combine weight
        sels.append(sel)
        wts.append(wt)
    maxc = max(len(s) for s in sels)
    ch = -(-maxc // (NCH * 8)) * 8       # chunk size, 8-aligned (16B in bf16)
    nch = NCH
    if ch > 512:                          # capacity > 1536 tokens: more chunks
        ch = 512
        nch = -(-maxc // ch)
    c = nch * ch

    in_maps = []
    for e in range(E):
        sel = sels[e]
        xe = np.zeros((c, D), f)
        xe[:len(sel)] = xf[sel]
        xTe = np.ascontiguousarray(
            xe.T.reshape(KD, P, c).transpose(1, 0, 2)).astype(NPBF16)
        w1te = np.ascontiguousarray(
            w1[e].reshape(KD, P, NH, P).transpose(2, 1, 0, 3)).astype(NPBF16)
        b1te = np.ascontiguousarray(b1[e].reshape(NH, P).T)
        w2te = np.ascontiguousarray(
            w2[e].reshape(NH, P, ND, P).transpose(2, 1, 0, 3)).astype(NPBF16)
        in_maps.append({"xT": xTe, "w1t": w1te, "b1t": b1te, "w2t": w2te})
    return in_maps, sels, wts, b2, nch, ch


def _assemble(results, sels, wts, b2):
    out = np.zeros((NTOK, D), np.float32)
    for e in range(E):
        sel = sels[e]
        yT = np.asarray(results[e]["outT"])                 # [P, ND, c] f32
        y = yT.transpose(1, 0, 2).reshape(D, -1).T          # [c, D]
        out[sel] += wts[e][:, None] * (y[:len(sel)] + b2[e])
    return out.reshape(2, NTOK // 2, D)


def run(inputs, trace=False):
    """Run the kernel; returns (output, exec_time_ns or None)."""
    in_maps, sels, wts, b2, nch, ch = _prep(**inputs)
    nc = _get_nc(nch, ch)
    res = bass_utils.run_bass_kernel_spmd(
        nc, in_maps, core_ids=list(range(NCORES)), trace=trace)
    return _assemble(res.results, sels, wts, b2), res.exec_time_ns


def kernel(**inputs):
    out, _ = run(inputs, trace=False)
    return out


# revision 16
# speedup vs baseline: 1.2273x; 1.0188x over previous
"""MoE layer (E=8 experts, top-2, D=1024, H=4096, N=4096 tokens) on 8 TRN2
NeuronCores.

Strategy: expert-parallel. The router (gate matmul + softmax + top-2) is a
negligible ~0.1% of the FLOPs, so it runs on host in float64 (verified to
reproduce the reference's f32 top-2 selection exactly). The host gathers
each expert's tokens, pads them to a common capacity C (max expert load
rounded up to the chunk size), and ships one expert per core. Each core
runs a dense single-expert FFN over its C tokens:

    outT = w2.T @ gelu(w1.T @ xT + b1)

i.e. 2*C*D*H MACs/core instead of the 8*2*T*D*H of the dense data-parallel
formulation — a ~3.6x FLOP cut (only top-2 experts per token are computed).
The host applies the combine weights and the (comb-weighted) b2 bias during
the scatter-add back to token order.

Activations keep the feature dim on partitions (tokens on the free axis) so
both matmuls consume pre-tiled weights with no on-device transposes:
    hT[h, t]   = sum_d w1[d, h] * xT[d, t]     (lhsT = w1 tile, rhs = xT)
    outT[d, t] = sum_h w2[h, d] * geluT[h, t]  (lhsT = w2 tile, rhs = geluT)

Matmuls run in bf16 (1 PE row/cycle, FWL-enabled weight loads) with fp32
PSUM accumulation; measured end-to-end error vs the f32 reference is
~3.4e-3 absmax-relative, well under the 2e-2 gate. Expected PE-bound time:
2 * 8 * 32 * C cycles @ 2.4 GHz ~= 246 us for C=1152.
"""

import numpy as np
import ml_dtypes

import concourse.bass as bass  # noqa: F401  (bass types used via tile/bacc)
import concourse.mybir as mybir
import concourse.tile as tile
from concourse import bacc, bass_utils

F32 = mybir.dt.float32
BF16 = mybir.dt.bfloat16
AFT = mybir.ActivationFunctionType
NPBF16 = np.dtype(ml_dtypes.bfloat16)

E = 8          # experts (== cores; expert e runs on core e)
D = 1024       # model dim
H = 4096       # expert hidden dim
P = 128        # partitions
NCORES = 8
NTOK = 4096    # total tokens (B*T = 2*2048)
KD = D // P    # 8 contraction chunks of D
NH = H // P    # 32 h tiles
ND = D // P    # 8 d tiles
NCH = 3        # token chunks per core (chunk = psum free dim, <= 512 f32/bank)

_NC = {}       # compiled modules keyed by (nch, ch)


def _build(nch, ch):
    c = nch * ch   # per-core token capacity
    nc = bacc.Bacc("TRN2", target_bir_lowering=False, debug=False,
                   num_devices=NCORES)
    xT = nc.dram_tensor("xT", [P, KD, c], BF16, kind="ExternalInput").ap()
    w1t = nc.dram_tensor("w1t", [NH, P, KD, P], BF16,
                         kind="ExternalInput").ap()
    b1t = nc.dram_tensor("b1t", [P, NH], F32, kind="ExternalInput").ap()
    w2t = nc.dram_tensor("w2t", [ND, P, NH, P], BF16,
                         kind="ExternalInput").ap()
    outT = nc.dram_tensor("outT", [P, ND, c], F32, kind="ExternalOutput").ap()

    with tile.TileContext(nc) as tc:
        with (
            tc.tile_pool(name="const", bufs=1) as cpool,
            tc.tile_pool(name="w1p", bufs=NH) as w1p,
            tc.tile_pool(name="w2p", bufs=2) as w2p,
            tc.tile_pool(name="otp", bufs=4) as otp,
            tc.tile_pool(name="ps", bufs=7, space="PSUM") as ps,
            tc.tile_pool(name="pw", bufs=1, space="PSUM") as pw,
        ):
            # ---- PE warmup: a short train of dependency-free matmuls keeps
            # the HAM activity window busy from ~0.5us so the real stream
            # starts at 2.4 GHz instead of ramping from 1.2 GHz. The train is
            # sized to end just before the first real matmul's data lands.
            wsrc = cpool.tile([P, 256], BF16)
            nc.vector.memset(wsrc[:], 0.0)
            pwt = pw.tile([P, P], F32)
            for _ in range(34):
                nc.tensor.matmul(pwt[:], wsrc[:, 0:P], wsrc[:, P:256],
                                 start=True, stop=True)

            # ---- persistent SBUF. DMA *instructions* issue serially at
            # ~0.65us each per issuing queue, so emission order and the
            # sync/scalar queue split set the time to first matmul: sync
            # issues b1+w1 tiles while scalar issues xT chunk columns. All
            # NH w1 tiles stay resident (64KB/partition) so mm1 can run
            # chunk-major: chunk 1/2 input data isn't touched until ~40us
            # in, by which time the xT fill has long completed.
            b1_s = cpool.tile([P, NH], F32)
            nc.sync.dma_start(b1_s[:], b1t[:])
            w1_all = []
            for h in range(NH):
                w1_s = w1p.tile([P, KD, P], BF16)
                nc.sync.dma_start(w1_s[:], w1t[h])
                w1_all.append(w1_s)
            xTt = cpool.tile([P, KD, c], BF16)
            for kd in range(KD):                      # chunk 0: needed first
                nc.scalar.dma_start(xTt[:, kd, 0:ch], xT[:, kd, 0:ch])
            for t in range(1, nch):                   # chunks 1+: issue after
                ts = slice(t * ch, (t + 1) * ch)      # the w1 fill (sync) so
                for kd in range(KD):                  # they don't contend
                    nc.sync.dma_start(xTt[:, kd, ts], xT[:, kd, ts])
            geluT = cpool.tile([P, NH, c], BF16)

            # ---- mm1 + gelu: geluT[h, t] = gelu(sum_d w1[d,h] x[d,t] + b1)
            for t in range(nch):
                ts = slice(t * ch, (t + 1) * ch)
                for h in range(NH):
                    ph = ps.tile([P, ch], F32, tag="ps")
                    for kd in range(KD):
                        nc.tensor.matmul(ph[:], w1_all[h][:, kd, :],
                                         xTt[:, kd, ts],
                                         start=(kd == 0), stop=(kd == KD - 1))
                    nc.scalar.activation(geluT[:, h, ts], ph[:], AFT.Gelu,
                                         bias=b1_s[:, h:h + 1])

            # ---- mm2: outT[d, t] = sum_h w2[h,d] geluT[h,t]
            for d in range(ND):
                w2_s = w2p.tile([P, NH, P], BF16)
                for q in range(2):
                    nc.sync.dma_start(
                        w2_s[:, q * NH // 2:(q + 1) * NH // 2, :],
                        w2t[d, :, q * NH // 2:(q + 1) * NH // 2, :])
                for t in range(nch):
                    ts = slice(t * ch, (t + 1) * ch)
                    po = ps.tile([P, ch], F32, tag="ps")
                    for hh in range(NH):
                        nc.tensor.matmul(po[:], w2_s[:, hh, :],
                                         geluT[:, hh, ts],
                                         start=(hh == 0), stop=(hh == NH - 1))
                    ot = otp.tile([P, ch], F32)
                    nc.vector.tensor_copy(ot[:], po[:])
                    nc.sync.dma_start(outT[:, d, ts], ot[:])

    nc.compile()
    return nc


def _get_nc(nch, ch):
    if (nch, ch) not in _NC:
        _NC[(nch, ch)] = _build(nch, ch)
    return _NC[(nch, ch)]


def _route(xf, gate_w, gate_b):
    """Top-2 routing in float64 (reproduces the reference's f32 decisions)."""
    lg = xf.astype(np.float64) @ gate_w.astype(np.float64) \
        + gate_b.astype(np.float64)
    lg -= lg.max(-1, keepdims=True)
    g = np.exp(lg)
    g /= g.sum(-1, keepdims=True)
    ti = np.argsort(-g, axis=-1, kind="stable")[:, :2]     # [N, 2] desc
    tg = np.take_along_axis(g, ti, axis=1)
    tg = tg / (tg.sum(-1, keepdims=True) + 1e-9)           # combine weights
    return ti, tg


def _prep(x, gate_w, gate_b, w1, b1, w2, b2):
    f = np.float32
    xf = np.asarray(x, f).reshape(NTOK, D)
    gate_w = np.asarray(gate_w, f)
    gate_b = np.asarray(gate_b, f)
    w1 = np.asarray(w1, f)
    b1 = np.asarray(b1, f)
    w2 = np.asarray(w2, f)
    b2 = np.asarray(b2, f)

    ti, tg = _route(xf, gate_w, gate_b)

    sels, wts = [], []
    for e in range(E):
        m = (ti == e)
        sel = np.nonzero(m.any(1))[0]                       # token ids, asc
        wt = tg[sel, m[sel].argmax(1)].astype(f)            # combine weight
        sels.append(sel)
        wts.append(wt)
    maxc = max(len(s) for s in sels)
    ch = -(-maxc // (NCH * 8)) * 8       # chunk size, 8-aligned (16B in bf16)
    nch = NCH
    if ch > 512:                          # capacity > 1536 tokens: more chunks
        ch = 512
        nch = -(-maxc // ch)
    c = nch * ch

    in_maps = []
    for e in range(E):
        sel = sels[e]
        xe = np.zeros((c, D), f)
        xe[:len(sel)] = xf[sel]
        xTe = np.ascontiguousarray(
            xe.T.reshape(KD, P, c).transpose(1, 0, 2)).astype(NPBF16)
        w1te = np.ascontiguousarray(
            w1[e].reshape(KD, P, NH, P).transpose(2, 1, 0, 3)).astype(NPBF16)
        b1te = np.ascontiguousarray(b1[e].reshape(NH, P).T)
        w2te = np.ascontiguousarray(
            w2[e].reshape(NH, P, ND, P).transpose(2, 1, 0, 3)).astype(NPBF16)
        in_maps.append({"xT": xTe, "w1t": w1te, "b1t": b1te, "w2t": w2te})
    return in_maps, sels, wts, b2, nch, ch


def _assemble(results, sels, wts, b2):
    out = np.zeros((NTOK, D), np.float32)
    for e in range(E):
        sel = sels[e]
        yT = np.asarray(results[e]["outT"])                 # [P, ND, c] f32
        y = yT.transpose(1, 0, 2).reshape(D, -1).T          # [c, D]
        out[sel] += wts[e][:, None] * (y[:len(sel)] + b2[e])
    return out.reshape(2, NTOK // 2, D)


def run(inputs, trace=False):
    """Run the kernel; returns (output, exec_time_ns or None)."""
    in_maps, sels, wts, b2, nch, ch = _prep(**inputs)
    nc = _get_nc(nch, ch)
    res = bass_utils.run_bass_kernel_spmd(
        nc, in_maps, core_ids=list(range(NCORES)), trace=trace)
    return _assemble(res.results, sels, wts, b2), res.exec_time_ns


def kernel(**inputs):
    out, _ = run(inputs, trace=False)
    return out
